# revision 5
# baseline (speedup 1.0000x reference)
"""Trainium2 Bass kernel for CKANConv2d (KAN conv: SiLU base + B-spline path).

Math: for each output pixel p and output channel co:
  out[co,p] = sum_{c,kh,kw} silu(x[c,p+k]) * Wb[co,(c,kh,kw)]
            + sum_{c,kh,kw,g} B_g(x[c,p+k]) * Ws[co,(c,kh,kw),g]
with B_g the order-3 uniform B-spline bases over knots {-2.2 + 0.4j}.

Key identity used on-chip (t = 2.5*x, center c_g = g - 3.5):
  v = |2.5 x - c_g|; m = min(v,2)-2; n = min(m+1,0)
  B_g(x) = (4 n^3 - m^3) / 6
The 1/6 is folded into the spline weights; the bases are computed per
*input* pixel (not per unfolded patch, 9x less work) and the 3x3
convolution is done as an implicit GEMM over 9 shifted windows with
contraction (c,g) packed 128 rows at a time.

Sharding: data-parallel over batch, 1 image per NeuronCore (8 cores).
"""
import numpy as np
import ml_dtypes

B, CIN, H, W = 8, 64, 56, 56
COUT, K = 128, 3
HO = WO = 54
NOUT = HO * WO  # 2916
NTAP = K * K  # 9
NKT = 4  # spline K-tiles per tap: 128 rows = 64c x 2g, 4 tiles cover g=0..7
NGRP = 6  # output row groups of 9 rows each
GROW = 9  # output rows per group
NFREE = GROW * WO  # 486 <= 512 (one PSUM bank)

_CACHE = {}


def _chunk_rows(j):
    """Input-row range (start, end) whose bases are computed in chunk j.
    Group r's matmuls read input rows [9r, 9r+10]; chunk j<=r covers them."""
    if j == 0:
        return 0, 11
    return 9 * j + 2, 9 * j + 11


def _patch_tile_tail_drain():
    """walrus in this env rejects the Tile tail Drain when it carries >1
    sync waits; split them into a chain of single-wait Drains."""
    import concourse.tile as tile
    from concourse.vector_clock import ScopedClock

    if getattr(tile.TileContext, "_drain_patched", False):
        return

    def _patched(self, tick_clock, wait_clock):
        drain_inst = self.nc.sync.drain()
        wait_clock.add_sem_waits(
            drain_inst.ins, ScopedClock({None: tick_clock.global_clock})
        )
        si = drain_inst.ins.sync_info
        waits = list(si.on_wait) if si is not None else []
        if len(waits) > 1:
            si.on_wait = waits[:1]
            handles = {h.num: h for h in self.sems.allocated().values()}
            for w in waits[1:]:
                extra = self.nc.sync.drain()
                extra.wait_op(handles[w.id], w.wait_value, "sem-ge")
        self.nc.all_engine_barrier()
        assert self.sems is not None
        popped = self.nc._tile_sem_poison_stack.pop()
        assert popped is self._sem_poison
        self.nc.clear_and_free_semaphores(list(self.sems.allocated().values()))
        self.nc.all_engine_barrier()

    tile.TileContext._drain_and_barrier = _patched
    tile.TileContext._drain_patched = True


def _split_excess_waits(nc, max_waits=1):
    """This walrus build encodes at most one sync-wait per instruction.
    Move extra waits onto same-engine NoOps inserted just before."""
    import bass_rust
    from concourse import mybir

    for f in nc.m.functions:
        for bb in f.blocks:
            new = []
            for ins in bb.instructions:
                si = ins.sync_info
                if si is not None and len(si.on_wait) > max_waits:
                    waits = list(si.on_wait)
                    for w in waits[: len(waits) - max_waits]:
                        nop = mybir.InstNoOp(
                            name=nc.get_next_instruction_name(), ins=[], outs=[]
                        )
                        nop.engine = ins.engine
                        h = bass_rust.SemaphoreHandle(name=w.ant_name, num=w.id)
                        bass_rust.wait_op(nop, h, w.wait_value, "sem-ge", False)
                        nc.register_instruction(nop, overwrite=True)
                        new.append(nop)
                    si.on_wait = waits[len(waits) - max_waits :]
                new.append(ins)
            bb.instructions = new


def _build():
    if "nc" in _CACHE:
        return _CACHE["nc"]
    _patch_tile_tail_drain()
    import concourse.bass as bass
    import concourse.tile as tile
    from concourse import mybir

    f32 = mybir.dt.float32
    bf16 = mybir.dt.bfloat16
    Alu = mybir.AluOpType
    Act = mybir.ActivationFunctionType

    nc = bass.Bass("TRN2")
    x_d = nc.dram_tensor("x", [CIN, H, W], f32, kind="ExternalInput").ap()
    wspl_d = nc.dram_tensor(
        "wspl", [128, NTAP * NKT, 128], bf16, kind="ExternalInput"
    ).ap()
    wbase_d = nc.dram_tensor("wbase", [CIN, NTAP, 128], bf16, kind="ExternalInput").ap()
    bneg_d = nc.dram_tensor("betaneg", [128, NKT], f32, kind="ExternalInput").ap()
    y_d = nc.dram_tensor("y", [128, HO, WO], f32, kind="ExternalOutput").ap()

    with tile.TileContext(nc) as tc:
        with (
            tc.tile_pool(name="consts", bufs=1) as cpool,
            tc.tile_pool(name="scratch", bufs=2) as spool,
            tc.tile_pool(name="psum", bufs=3, space="PSUM") as ppool,
        ):
            wspl = cpool.tile([128, NTAP * NKT, 128], bf16, tag="wspl")
            nc.sync.dma_start(wspl[:], wspl_d)
            wbase = cpool.tile([CIN, NTAP, 128], bf16, tag="wbase")
            nc.sync.dma_start(wbase[:], wbase_d)
            bneg = cpool.tile([128, NKT], f32, tag="bneg")
            nc.sync.dma_start(bneg[:], bneg_d)

            x2 = cpool.tile([128, H, W], f32, tag="x2")
            nc.sync.dma_start(x2[0:CIN, :, :], x_d)
            nc.sync.dma_start(x2[CIN:128, :, :], x_d)

            silu2 = cpool.tile([128, H, W], bf16, tag="silu2")
            nc.scalar.activation(silu2[:], x2[:], Act.Silu)

            rhs = [
                cpool.tile([128, H, W], bf16, tag=f"rhs{t}", name=f"rhs{t}")
                for t in range(NKT)
            ]

            def elementwise(t, r0, r1):
                rows = r1 - r0
                xs = x2[:, r0:r1, :]
                v = spool.tile([128, rows, W], f32, tag="v")
                nc.scalar.activation(
                    v[:], xs, Act.Abs, bias=bneg[:, t : t + 1], scale=2.5
                )
                m = spool.tile([128, rows, W], f32, tag="m")
                nc.vector.tensor_scalar(m[:], v[:], 2.0, 2.0, Alu.min, Alu.subtract)
                n = spool.tile([128, rows, W], f32, tag="n")
                nc.vector.tensor_scalar(n[:], v[:], 1.0, 1.0, Alu.min, Alu.subtract)
                m2 = spool.tile([128, rows, W], f32, tag="m2")
                nc.scalar.activation(m2[:], m[:], Act.Square)
                n2 = spool.tile([128, rows, W], f32, tag="n2")
                nc.scalar.activation(n2[:], n[:], Act.Square)
                s1 = spool.tile([128, rows, W], f32, tag="s1")
                nc.vector.scalar_tensor_tensor(
                    s1[:], m2[:], -1.0, m[:], Alu.mult, Alu.mult
                )
                s2 = spool.tile([128, rows, W], f32, tag="s2")
                nc.vector.scalar_tensor_tensor(
                    s2[:], n2[:], 4.0, n[:], Alu.mult, Alu.mult
                )
                nc.vector.tensor_tensor(rhs[t][:, r0:r1, :], s1[:], s2[:], Alu.add)

            for grp in range(NGRP):
                r0, r1 = _chunk_rows(grp)
                for t in range(NKT):
                    elementwise(t, r0, r1)

                ps = ppool.tile([128, NFREE], f32, tag="ps")
                k = 0
                nmm = NTAP * (NKT + 1)
                for tap in range(NTAP):
                    kh, kw = divmod(tap, K)
                    for t in range(NKT):
                        rv = rhs[t][:, 9 * grp + kh : 9 * grp + kh + GROW, kw : kw + WO]
                        nc.tensor.matmul(
                            ps[:],
                            wspl[:, tap * NKT + t, :],
                            rv,
                            start=(k == 0),
                            stop=(k == nmm - 1),
                        )
                        k += 1
                    sv = silu2[0:CIN, 9 * grp + kh : 9 * grp + kh + GROW, kw : kw + WO]
                    nc.tensor.matmul(
                        ps[:],
                        wbase[0:CIN, tap, :],
                        sv,
                        start=(k == 0),
                        stop=(k == nmm - 1),
                    )
                    k += 1

                ev = spool.tile([128, NFREE], f32, tag="ev")
                nc.scalar.copy(ev[:], ps[:])
                nc.sync.dma_start(y_d[:, GROW * grp : GROW * (grp + 1), :], ev[:])

    _split_excess_waits(nc)
    _CACHE["nc"] = nc
    return nc


def _prep_weights(base_weight, spline_weight, spline_scaler):
    """Fold scaler and 1/6 into spline weights; lay out matmul lhsT tiles."""
    sw = (spline_weight * spline_scaler[:, :, None]).astype(np.float32) / 6.0
    # sw: [COUT, 576, 8]; feature index i = c*9 + tap
    sw4 = sw.reshape(COUT, CIN, NTAP, 8)  # [co, c, tap, g]
    # wspl[p, tap*4+t, co] = sw4[co, c, tap, 2t+gh], p = gh*64 + c
    w = np.transpose(sw4, (1, 2, 3, 0))  # [c, tap, g, co]
    # -> [gh, c, tap, t, co] with g = 2t + gh
    w = w.reshape(CIN, NTAP, NKT, 2, COUT)  # g = 2t + gh -> [c, tap, t, gh, co]
    w = np.transpose(w, (3, 0, 1, 2, 4))  # [gh, c, tap, t, co]
    wspl = w.reshape(2 * CIN, NTAP * NKT, COUT).astype(ml_dtypes.bfloat16)

    wb = base_weight.reshape(COUT, CIN, NTAP)  # [co, c, tap]
    wbase = np.transpose(wb, (1, 2, 0)).astype(ml_dtypes.bfloat16)  # [c, tap, co]

    gh = np.arange(128) // CIN  # 0 for p<64, 1 otherwise
    t = np.arange(NKT)
    bneg = (3.5 - (2 * t[None, :] + gh[:, None])).astype(np.float32)  # [128, 4]
    return wspl, wbase, bneg


def _in_maps(x, base_weight, spline_weight, spline_scaler):
    wspl, wbase, bneg = _prep_weights(base_weight, spline_weight, spline_scaler)
    return [
        {
            "x": np.ascontiguousarray(x[b]).astype(np.float32),
            "wspl": wspl,
            "wbase": wbase,
            "betaneg": bneg,
        }
        for b in range(B)
    ]


def kernel(x, base_weight, spline_weight, spline_scaler):
    from concourse.bass_utils import run_bass_kernel_spmd

    nc = _build()
    in_maps = _in_maps(x, base_weight, spline_weight, spline_scaler)
    res = run_bass_kernel_spmd(nc, in_maps, core_ids=list(range(B)))
    out = np.stack([res.results[b]["y"] for b in range(B)])  # [8, 128, 54, 54]
    return out.astype(np.float32)
